# revision 1
# baseline (speedup 1.0000x reference)
"""Trainium2 Bass kernel for nn_ClassicalSelfAttention (B=4, S=2048, E=1024).

Reference computation (fp32):
    w_qkv = rotation_params.reshape(3E, E); w_out = entangle_params.reshape(E, E)
    qkv = x @ w_qkv.T; q, k, v = split(qkv)
    scores = (q / sqrt(64)) @ k.T          # full-E attention, no heads
    attn = softmax(scores, axis=-1)
    out = (attn @ v) @ w_out.T
    result = sigmoid(out @ gate_w.T) * out

Sharding: 8 cores = 4 batches x 2 query-halves. Each core computes K/V for its
whole batch (duplicated within the pair) and attention + projections for its
1024 queries. Key order is rotated per query-half so each core's queries are
always columns 0:1024 of its (host-pre-transposed) x^T input — softmax and
attn@v are permutation-invariant in key order.

All heavy matmuls run in float32r (fp32 with 11-bit mantissa, full PE speed at
free-dim 512). Data layout is feature-major ("transposed") throughout:
    xT [e, s] -> qT [f, s], kT [f, s] (moving/stationary for scores)
               -> v [s, f] natural (stationary for attn@v)
    scores [qi, kj] -> softmax along free dim -> normalized attn
    PE-transpose attn -> attnT [kj, qi]
    attn_outT [e, qi] = v.T @ attnT
    outT [f, qi] = w_outT.T @ attn_outT
    gateT [f', qi] = gw.T.T @ outT;  result^T = sigmoid(gateT) * outT
Host untransposes the per-core [E, 1024] result tiles.
"""

from contextlib import ExitStack

import numpy as np

import concourse.bass as bass
import concourse.tile as tile
from concourse import bacc, mybir
from concourse.bass_utils import run_bass_kernel_spmd
from concourse.masks import make_identity

F32 = mybir.dt.float32
F32R = mybir.dt.float32r

P = 128
E = 1024
B = 4
S = 2048
SK = S            # keys per core (full batch sequence)
SQ = S // 2       # queries per core (half)
ET = E // P       # 8 e-tiles
KT = SK // P      # 16 key tiles
NC = 512          # moving-operand chunk (f32r full speed needs >=256, max 512)
SKC = SK // NC    # 4
SQC = SQ // NC    # 2
FC = E // NC      # 2
NCORES = 8
SCALE = 1.0 / 8.0  # 1/sqrt(head_dim=64), folded into exp()


def _round_fp32r(x: np.ndarray) -> np.ndarray:
    """Round-to-nearest-even to fp32r (11-bit mantissa; low 12 bits zero)."""
    u = np.ascontiguousarray(x, dtype=np.float32).view(np.uint32).astype(np.uint64)
    r = (u + 0x7FF + ((u >> 12) & 1)) & ~np.uint64(0xFFF)
    return r.astype(np.uint32).view(np.float32)


def _build_nc():
    nc = bacc.Bacc("TRN2", target_bir_lowering=False, debug=False,
                   num_devices=NCORES)
    xT = nc.dram_tensor("xT", [E, SK], F32R, kind="ExternalInput").ap()
    wqT = nc.dram_tensor("wqT", [E, E], F32R, kind="ExternalInput").ap()
    wkT = nc.dram_tensor("wkT", [E, E], F32R, kind="ExternalInput").ap()
    wvT = nc.dram_tensor("wvT", [E, E], F32R, kind="ExternalInput").ap()
    woT = nc.dram_tensor("woT", [E, E], F32R, kind="ExternalInput").ap()
    gwT = nc.dram_tensor("gwT", [E, E], F32R, kind="ExternalInput").ap()
    outT = nc.dram_tensor("outT", [E, SQ], F32, kind="ExternalOutput").ap()

    with tile.TileContext(nc) as tc, ExitStack() as ctx:
        _emit(tc, ctx, xT, wqT, wkT, wvT, woT, gwT, outT)
    nc.compile()
    return nc


def _emit(tc, ctx, xT, wqT, wkT, wvT, woT, gwT, outT):
    nc = tc.nc
    Exp = mybir.ActivationFunctionType.Exp
    Sigmoid = mybir.ActivationFunctionType.Sigmoid

    singles = ctx.enter_context(tc.tile_pool(name="singles", bufs=1))
    ident_f = singles.tile([P, P], F32, tag="ident_f")
    make_identity(nc, ident_f)
    ident = singles.tile([P, P], F32R, tag="ident")
    nc.vector.tensor_copy(out=ident[:], in_=ident_f[:])

    dram = ctx.enter_context(tc.tile_pool(name="dram", bufs=1, space="DRAM"))
    vsp = dram.tile([SK, E], F32R, tag="vsp")

    # Staged-lifetime SBUF pools (overlapping, hence explicit ExitStacks):
    #   qt/kt: phase1 -> end of 2a;  att: 2a -> end of 2b;  aot: 2b -> end.
    ps_mm = ctx.enter_context(tc.tile_pool(name="ps_mm", bufs=6, space="PSUM"))

    es_qk = ExitStack()
    qt_pool = es_qk.enter_context(tc.tile_pool(name="qt", bufs=1))
    kt_pool = es_qk.enter_context(tc.tile_pool(name="kt", bufs=1))
    qt = [qt_pool.tile([P, SQ], F32R, tag=f"qt{i}", name=f"qt{i}") for i in range(ET)]
    kt = [kt_pool.tile([P, SK], F32R, tag=f"kt{i}", name=f"kt{i}") for i in range(ET)]

    # ---------------- Phase 1: qT, v (spilled), kT ----------------
    with tc.tile_pool(name="xt", bufs=1) as xt_pool, \
         tc.tile_pool(name="wp", bufs=1) as w_pool, \
         tc.tile_pool(name="vb", bufs=2) as vb_pool:

        # interleaved loads: qT (first compute) needs only wq + xT cols 0:SQ
        xt, wq = [], []
        for et in range(ET):
            tw = w_pool.tile([P, E], F32R, tag=f"w{et}", name=f"wq{et}")
            nc.sync.dma_start(out=tw[:], in_=wqT[et * P:(et + 1) * P, :])
            wq.append(tw)
            t = xt_pool.tile([P, SK], F32R, tag=f"xt{et}", name=f"xt{et}")
            nc.sync.dma_start(out=t[:, 0:SQ], in_=xT[et * P:(et + 1) * P, 0:SQ])
            xt.append(t)
        # --- qT[f, s] = wqT.T @ xq
        for ft in range(ET):
            psums = [ps_mm.tile([P, NC], F32, tag="mm", name="mmp") for _ in range(SQC)]
            for et in range(ET):
                for sc in range(SQC):
                    nc.tensor.matmul(
                        psums[sc][:],
                        wq[et][:, ft * P:(ft + 1) * P],
                        xt[et][:, sc * NC:(sc + 1) * NC],
                        start=(et == 0), stop=(et == ET - 1),
                    )
            for sc in range(SQC):
                nc.vector.tensor_copy(
                    out=qt[ft][:, sc * NC:(sc + 1) * NC], in_=psums[sc][:])

        # --- v[s, f] = xT.T @ wvT : stationary xT block, moving wv; spill to DRAM
        wv = []
        for et in range(ET):
            t = w_pool.tile([P, E], F32R, tag=f"w{et}")
            nc.sync.dma_start(out=t[:], in_=wvT[et * P:(et + 1) * P, :])
            wv.append(t)
            nc.sync.dma_start(
                out=xt[et][:, SQ:SK], in_=xT[et * P:(et + 1) * P, SQ:SK])
        for st in range(KT):
            psums = [ps_mm.tile([P, NC], F32, tag="mm", name="mmp") for _ in range(FC)]
            for et in range(ET):
                for fc in range(FC):
                    nc.tensor.matmul(
                        psums[fc][:],
                        xt[et][:, st * P:(st + 1) * P],
                        wv[et][:, fc * NC:(fc + 1) * NC],
                        start=(et == 0), stop=(et == ET - 1),
                    )
            vb = vb_pool.tile([P, E], F32R, tag="vb")
            for fc in range(FC):
                nc.vector.tensor_copy(
                    out=vb[:, fc * NC:(fc + 1) * NC], in_=psums[fc][:])
            nc.sync.dma_start(out=vsp[st * P:(st + 1) * P, :], in_=vb[:])

        # --- kT[f, s] = wkT.T @ xT (full SK columns)
        wk = []
        for et in range(ET):
            t = w_pool.tile([P, E], F32R, tag=f"w{et}")
            nc.sync.dma_start(out=t[:], in_=wkT[et * P:(et + 1) * P, :])
            wk.append(t)
        for ft in range(ET):
            psums = [ps_mm.tile([P, NC], F32, tag="mm", name="mmp") for _ in range(SKC)]
            for et in range(ET):
                for kc in range(SKC):
                    nc.tensor.matmul(
                        psums[kc][:],
                        wk[et][:, ft * P:(ft + 1) * P],
                        xt[et][:, kc * NC:(kc + 1) * NC],
                        start=(et == 0), stop=(et == ET - 1),
                    )
            for kc in range(SKC):
                nc.vector.tensor_copy(
                    out=kt[ft][:, kc * NC:(kc + 1) * NC], in_=psums[kc][:])

    # ---------------- Phase 2a: scores -> softmax -> attnT ----------------
    es_att = ExitStack()
    att_pool = es_att.enter_context(tc.tile_pool(name="att", bufs=1, side="right"))
    att = [att_pool.tile([P, SQ], F32R, tag=f"at{i}", name=f"at{i}") for i in range(KT)]

    with tc.tile_pool(name="exp", bufs=2) as exp_pool, \
         tc.tile_pool(name="sums", bufs=4) as sums_pool, \
         tc.tile_pool(name="ps_t", bufs=2, space="PSUM") as ps_t:

        for sb in range(ET):  # 8 query sub-blocks of 128
            psums = [ps_mm.tile([P, NC], F32, tag="mm", name="mmp") for _ in range(SKC)]
            for et in range(ET):
                for kc in range(SKC):
                    nc.tensor.matmul(
                        psums[kc][:],
                        qt[et][:, sb * P:(sb + 1) * P],
                        kt[et][:, kc * NC:(kc + 1) * NC],
                        start=(et == 0), stop=(et == ET - 1),
                    )
            exp_t = exp_pool.tile([P, SK], F32, tag="exp")
            sums4 = sums_pool.tile([P, SKC], F32, tag="sums4")
            for kc in range(SKC):
                nc.scalar.activation(
                    out=exp_t[:, kc * NC:(kc + 1) * NC],
                    in_=psums[kc][:], func=Exp, scale=SCALE,
                    accum_out=sums4[:, kc:kc + 1],
                )
            sum1 = sums_pool.tile([P, 1], F32, tag="sum1")
            nc.vector.tensor_reduce(
                out=sum1[:], in_=sums4[:],
                axis=mybir.AxisListType.X, op=mybir.AluOpType.add)
            recip = sums_pool.tile([P, 1], F32, tag="recip")
            nc.vector.reciprocal(out=recip[:], in_=sum1[:])
            attn_n = exp_pool.tile([P, SK], F32R, tag="attn_n", bufs=2)
            nc.scalar.mul(out=attn_n[:], in_=exp_t[:], mul=recip[:])
            for kj in range(KT):
                pst = ps_t.tile([P, P], F32R, tag="pst")
                nc.tensor.transpose(
                    pst[:], attn_n[:, kj * P:(kj + 1) * P], ident[:])
                nc.vector.tensor_copy(
                    out=att[kj][:, sb * P:(sb + 1) * P], in_=pst[:])

    # ---------------- Phase 2b: attn_outT[e, qi] = v.T @ attnT ----------------
    es_qk.close()  # qt/kt freed after scores
    aot_pool = ctx.enter_context(tc.tile_pool(name="aot", bufs=1))
    aot = [aot_pool.tile([P, SQ], F32R, tag=f"ao{i}", name=f"ao{i}") for i in range(ET)]

    w2_es = ExitStack()
    w2_pool = w2_es.enter_context(tc.tile_pool(name="wp2", bufs=1))
    with tc.tile_pool(name="vt", bufs=1) as v_pool:
        vt = []
        for st in range(KT):
            t = v_pool.tile([P, E], F32R, tag=f"v{st}", name=f"v{st}")
            nc.sync.dma_start(out=t[:], in_=vsp[st * P:(st + 1) * P, :])
            vt.append(t)
        # prefetch out-projection weights during attn@v
        wo = []
        for et in range(ET):
            t = w2_pool.tile([P, E], F32R, tag=f"w2{et}", name=f"wo{et}")
            nc.sync.dma_start(out=t[:], in_=woT[et * P:(et + 1) * P, :])
            wo.append(t)
        for et in range(ET):
            psums = [ps_mm.tile([P, NC], F32, tag="mm", name="mmp") for _ in range(SQC)]
            for kj in range(KT):
                for qc in range(SQC):
                    nc.tensor.matmul(
                        psums[qc][:],
                        vt[kj][:, et * P:(et + 1) * P],
                        att[kj][:, qc * NC:(qc + 1) * NC],
                        start=(kj == 0), stop=(kj == KT - 1),
                    )
            for qc in range(SQC):
                nc.vector.tensor_copy(
                    out=aot[et][:, qc * NC:(qc + 1) * NC], in_=psums[qc][:])

    es_att.close()  # att freed after attn@v

    # ---------------- Phase 2c: outT, gate, result ----------------
    with tc.tile_pool(name="ot", bufs=1) as ot_pool, \
         tc.tile_pool(name="fin", bufs=2) as fin_pool:

        ot = [ot_pool.tile([P, SQ], F32R, tag=f"ot{i}", name=f"ot{i}") for i in range(ET)]
        for ft in range(ET):
            psums = [ps_mm.tile([P, NC], F32, tag="mm", name="mmp") for _ in range(SQC)]
            for et in range(ET):
                for qc in range(SQC):
                    nc.tensor.matmul(
                        psums[qc][:],
                        wo[et][:, ft * P:(ft + 1) * P],
                        aot[et][:, qc * NC:(qc + 1) * NC],
                        start=(et == 0), stop=(et == ET - 1),
                    )
            for qc in range(SQC):
                nc.vector.tensor_copy(
                    out=ot[ft][:, qc * NC:(qc + 1) * NC], in_=psums[qc][:])

        gw = []
        for et in range(ET):
            t = w2_pool.tile([P, E], F32R, tag=f"w2{et}")
            nc.sync.dma_start(out=t[:], in_=gwT[et * P:(et + 1) * P, :])
            gw.append(t)
        for ft in range(ET):
            psums = [ps_mm.tile([P, NC], F32, tag="mm", name="mmp") for _ in range(SQC)]
            for et in range(ET):
                for qc in range(SQC):
                    nc.tensor.matmul(
                        psums[qc][:],
                        gw[et][:, ft * P:(ft + 1) * P],
                        ot[et][:, qc * NC:(qc + 1) * NC],
                        start=(et == 0), stop=(et == ET - 1),
                    )
            fin = fin_pool.tile([P, SQ], F32, tag="fin")
            for qc in range(SQC):
                gate = fin_pool.tile([P, NC], F32, tag="gate")
                nc.scalar.activation(
                    out=gate[:], in_=psums[qc][:], func=Sigmoid)
                nc.vector.tensor_mul(
                    fin[:, qc * NC:(qc + 1) * NC], gate[:],
                    ot[ft][:, qc * NC:(qc + 1) * NC].bitcast(F32))
            nc.sync.dma_start(out=outT[ft * P:(ft + 1) * P, :], in_=fin[:])

    w2_es.close()


_NC_CACHE = None


def _get_nc():
    global _NC_CACHE
    if _NC_CACHE is None:
        _NC_CACHE = _build_nc()
    return _NC_CACHE


def _prep_in_maps(rotation_params, entangle_params, inputs, gate_w):
    w_qkv = np.asarray(rotation_params, dtype=np.float32).reshape(3 * E, E)
    wq, wk, wv = w_qkv[:E], w_qkv[E:2 * E], w_qkv[2 * E:]
    w_out = np.asarray(entangle_params, dtype=np.float32).reshape(E, E)
    gw = np.asarray(gate_w, dtype=np.float32)
    x = np.asarray(inputs, dtype=np.float32)

    wqT = _round_fp32r(wq.T)
    wkT = _round_fp32r(wk.T)
    wvT = _round_fp32r(wv.T)
    woT = _round_fp32r(w_out.T)
    gwT = _round_fp32r(gw.T)

    in_maps = []
    for c in range(NCORES):
        b, h = c // 2, c % 2
        xT = x[b].T  # [E, S]
        if h == 1:   # rotate keys so this core's queries sit at columns 0:SQ
            xT = np.concatenate([xT[:, SQ:], xT[:, :SQ]], axis=1)
        in_maps.append({
            "xT": _round_fp32r(xT),
            "wqT": wqT, "wkT": wkT, "wvT": wvT, "woT": woT, "gwT": gwT,
        })
    return in_maps


def _assemble(results):
    out = np.empty((B, S, E), dtype=np.float32)
    for c in range(NCORES):
        b, h = c // 2, c % 2
        out[b, h * SQ:(h + 1) * SQ, :] = results[c]["outT"].T
    return out


def _run(in_maps, trace=False):
    nc = _get_nc()
    return run_bass_kernel_spmd(nc, in_maps, core_ids=list(range(NCORES)),
                                trace=trace)


def kernel(rotation_params, entangle_params, inputs, gate_w):
    in_maps = _prep_in_maps(rotation_params, entangle_params, inputs, gate_w)
    res = _run(in_maps, trace=False)
    return _assemble(res.results)



# revision 7
# speedup vs baseline: 1.1375x; 1.1375x over previous
"""Trainium2 Bass kernel for nn_ClassicalSelfAttention (B=4, S=2048, E=1024).

Reference computation (fp32):
    w_qkv = rotation_params.reshape(3E, E); w_out = entangle_params.reshape(E, E)
    qkv = x @ w_qkv.T; q, k, v = split(qkv)
    scores = (q / sqrt(64)) @ k.T          # full-E attention, no heads
    attn = softmax(scores, axis=-1)
    out = (attn @ v) @ w_out.T
    result = sigmoid(out @ gate_w.T) * out

Sharding: 8 cores = 4 batches x 2 query-halves. Each core computes K/V for its
whole batch (duplicated within the pair) and attention + projections for its
1024 queries. Key order is rotated per query-half so each core's queries are
always columns 0:1024 of its (host-pre-transposed) x^T input — softmax and
attn@v are permutation-invariant in key order.

All matmul operands are bf16 (1 col/cycle on the PE, same as f32r, but half
the DMA/SBUF), accumulation in fp32 PSUM. Scores are computed TRANSPOSED
([kj, qi]) so exp() lands directly in the layout attn@v needs — no PE
transposes. Softmax denominators come from an all-ones stationary matmul
(which broadcasts the per-query sums across all 128 partitions for free);
normalization is deferred past attn@v (it is a per-query-column scale, which
commutes with contraction over keys) and applied on the PSUM->SBUF copy.

Layout (feature-major throughout):
    xT [e, s] -> qT [f, s], kT [f, s], v [s, f] (resident in SBUF, no spill)
    scoresT [kj, qi] = kT.T @ qT -> exp -> attT [kj, qi] (bf16)
    sums_bcast [*, qi] = ones.T @ attT (PSUM accum over kj); recip = 1/sums
    aotT [e, qi] = (v.T @ attT) * recip
    outT [f, qi] = woT.T @ aotT;  gateT = gwT.T @ outT
    resultT = sigmoid(gateT) * outT
Host untransposes the per-core [E, 1024] result tiles.
"""

from contextlib import ExitStack

import numpy as np
import ml_dtypes

import concourse.bass as bass
import concourse.tile as tile
from concourse import bacc, mybir
from concourse.bass_utils import run_bass_kernel_spmd

F32 = mybir.dt.float32
BF16 = mybir.dt.bfloat16

P = 128
E = 1024
B = 4
S = 2048
SK = S            # keys per core (full batch sequence)
SQ = S // 2       # queries per core (half)
ET = E // P       # 8 e-tiles
KT = SK // P      # 16 key tiles
NC = 512          # moving-operand chunk (max free dim per PSUM bank)
SKC = SK // NC    # 4
SQC = SQ // NC    # 2
FC = E // NC      # 2
NCORES = 8
SCALE = 1.0 / 8.0  # 1/sqrt(head_dim=64), folded into exp()


def _build_nc():
    nc = bacc.Bacc("TRN2", target_bir_lowering=False, debug=False,
                   num_devices=NCORES)
    xT = nc.dram_tensor("xT", [E, SK], BF16, kind="ExternalInput").ap()
    wqT = nc.dram_tensor("wqT", [E, E], BF16, kind="ExternalInput").ap()
    wkT = nc.dram_tensor("wkT", [E, E], BF16, kind="ExternalInput").ap()
    wvT = nc.dram_tensor("wvT", [E, E], BF16, kind="ExternalInput").ap()
    woT = nc.dram_tensor("woT", [E, E], BF16, kind="ExternalInput").ap()
    gwT = nc.dram_tensor("gwT", [E, E], BF16, kind="ExternalInput").ap()
    outT = nc.dram_tensor("outT", [E, SQ], F32, kind="ExternalOutput").ap()

    with tile.TileContext(nc) as tc, ExitStack() as ctx:
        _emit(tc, ctx, xT, wqT, wkT, wvT, woT, gwT, outT)
    nc.compile()
    return nc


def _emit(tc, ctx, xT, wqT, wkT, wvT, woT, gwT, outT):
    nc = tc.nc
    Exp = mybir.ActivationFunctionType.Exp
    Sigmoid = mybir.ActivationFunctionType.Sigmoid

    singles = ctx.enter_context(tc.tile_pool(name="singles", bufs=1))
    ones = singles.tile([P, P], BF16, tag="ones")
    nc.gpsimd.memset(ones[:], 1.0)

    ps_mm = ctx.enter_context(tc.tile_pool(name="ps_mm", bufs=6, space="PSUM"))

    # Pool stacks are LIFO per side. Left, bottom-up: singles, nrm, v, aot
    # (all ctx-lifetime), then qt/kt (closed after 2a), then phase-1 scratch
    # (closed after phase 1), then 2c scratch. Right: wp2 (ctx-lifetime),
    # then att (closed after 2b).
    nrm_pool = ctx.enter_context(tc.tile_pool(name="nrm", bufs=1))
    recip = nrm_pool.tile([P, SQ], F32, tag="recip")
    v_pool = ctx.enter_context(tc.tile_pool(name="vt", bufs=1))
    vt = [v_pool.tile([P, E], BF16, tag=f"v{i}", name=f"v{i}") for i in range(KT)]
    aot_pool = ctx.enter_context(tc.tile_pool(name="aot", bufs=1))

    es_qk = ExitStack()
    qt_pool = es_qk.enter_context(tc.tile_pool(name="qt", bufs=1))
    kt_pool = es_qk.enter_context(tc.tile_pool(name="kt", bufs=1))
    qt = [qt_pool.tile([P, SQ], BF16, tag=f"qt{i}", name=f"qt{i}") for i in range(ET)]
    kt = [kt_pool.tile([P, SK], BF16, tag=f"kt{i}", name=f"kt{i}") for i in range(ET)]

    # ---------------- Phase 1: qT, v (resident), kT ----------------
    with tc.tile_pool(name="xt", bufs=1) as xt_pool, \
         tc.tile_pool(name="wp", bufs=1) as w_pool:

        # interleaved loads, finest-need-first: the first qT group touches
        # only columns 0:256 of each wq tile and columns 0:512 of each xt.
        xt = [xt_pool.tile([P, SK], BF16, tag=f"xt{et}", name=f"xt{et}")
              for et in range(ET)]
        wq = [w_pool.tile([P, E], BF16, tag=f"w{et}", name=f"wq{et}")
              for et in range(ET)]
        for fh in range(4):
            for et in range(ET):
                nc.sync.dma_start(
                    out=wq[et][:, fh * 256:(fh + 1) * 256],
                    in_=wqT[et * P:(et + 1) * P, fh * 256:(fh + 1) * 256])
            if fh < 2:
                for et in range(ET):
                    nc.sync.dma_start(
                        out=xt[et][:, fh * NC:(fh + 1) * NC],
                        in_=xT[et * P:(et + 1) * P, fh * NC:(fh + 1) * NC])

        # --- qT[f, s] = wqT.T @ xq : groups of 2 ft, accumulate over et
        for sc in range(SQC):
            for fh in range(4):
                psums = [ps_mm.tile([P, NC], F32, tag="mm", name="mmp")
                         for _ in range(2)]
                for et in range(ET):
                    for f2 in range(2):
                        ft = fh * 2 + f2
                        nc.tensor.matmul(
                            psums[f2][:],
                            wq[et][:, ft * P:(ft + 1) * P],
                            xt[et][:, sc * NC:(sc + 1) * NC],
                            start=(et == 0), stop=(et == ET - 1),
                        )
                for f2 in range(2):
                    ft = fh * 2 + f2
                    nc.vector.tensor_copy(
                        out=qt[ft][:, sc * NC:(sc + 1) * NC], in_=psums[f2][:])

        # --- v[s, f] = xT.T @ wvT : stationary xT block, moving wv; resident
        wv = []
        for et in range(ET):
            t = w_pool.tile([P, E], BF16, tag=f"wv{et}", name=f"wv{et}")
            nc.sync.dma_start(out=t[:], in_=wvT[et * P:(et + 1) * P, :])
            wv.append(t)
            nc.sync.dma_start(
                out=xt[et][:, SQ:SK], in_=xT[et * P:(et + 1) * P, SQ:SK])
        wk = []
        for et in range(ET):
            t = w_pool.tile([P, E], BF16, tag=f"wk{et}", name=f"wk{et}")
            nc.sync.dma_start(out=t[:], in_=wkT[et * P:(et + 1) * P, :])
            wk.append(t)
        for st in range(KT):
            psums = [ps_mm.tile([P, NC], F32, tag="mm", name="mmp")
                     for _ in range(FC)]
            for et in range(ET):
                for fc in range(FC):
                    nc.tensor.matmul(
                        psums[fc][:],
                        xt[et][:, st * P:(st + 1) * P],
                        wv[et][:, fc * NC:(fc + 1) * NC],
                        start=(et == 0), stop=(et == ET - 1),
                    )
            for fc in range(FC):
                nc.vector.tensor_copy(
                    out=vt[st][:, fc * NC:(fc + 1) * NC], in_=psums[fc][:])

        # --- kT[f, s] = wkT.T @ xT (full SK columns), groups of 2 kc
        for ft in range(ET):
            for kh in range(2):
                psums = [ps_mm.tile([P, NC], F32, tag="mm", name="mmp")
                         for _ in range(2)]
                for et in range(ET):
                    for k2 in range(2):
                        kc = kh * 2 + k2
                        nc.tensor.matmul(
                            psums[k2][:],
                            wk[et][:, ft * P:(ft + 1) * P],
                            xt[et][:, kc * NC:(kc + 1) * NC],
                            start=(et == 0), stop=(et == ET - 1),
                        )
                for k2 in range(2):
                    kc = kh * 2 + k2
                    nc.vector.tensor_copy(
                        out=kt[ft][:, kc * NC:(kc + 1) * NC], in_=psums[k2][:])

    # ---------------- Phase 2a: scoresT -> exp -> attT; sums via ones ----------------
    w2_es = ExitStack()
    w2_pool = w2_es.enter_context(tc.tile_pool(name="wp2", bufs=1, side="right"))
    wo = []
    for et in range(ET):
        t = w2_pool.tile([P, E], BF16, tag=f"w2{et}", name=f"wo{et}")
        nc.sync.dma_start(out=t[:], in_=woT[et * P:(et + 1) * P, :])
        wo.append(t)

    es_att = ExitStack()
    att_pool = es_att.enter_context(tc.tile_pool(name="att", bufs=1, side="right"))
    att = [att_pool.tile([P, SQ], BF16, tag=f"at{i}", name=f"at{i}")
           for i in range(KT)]

    sums_pool = ctx.enter_context(tc.tile_pool(name="ps_sums", bufs=1, space="PSUM"))
    sums_ps = [sums_pool.tile([P, NC], F32, tag=f"sums{sc}", name=f"sums{sc}")
               for sc in range(SQC)]

    def emit_sums(kj):
        # per-query exp-sums, broadcast to all 128 partitions by all-ones
        # stationary; PSUM-accumulated across all 16 key tiles.
        for sc in range(SQC):
            nc.tensor.matmul(
                sums_ps[sc][:],
                ones[:],
                att[kj][:, sc * NC:(sc + 1) * NC],
                start=(kj == 0), stop=(kj == KT - 1),
            )

    for kj in range(KT):
        psums = [ps_mm.tile([P, NC], F32, tag="mm", name="mmp")
                 for _ in range(SQC)]
        for et in range(ET):
            for sc in range(SQC):
                nc.tensor.matmul(
                    psums[sc][:],
                    kt[et][:, kj * P:(kj + 1) * P],
                    qt[et][:, sc * NC:(sc + 1) * NC],
                    start=(et == 0), stop=(et == ET - 1),
                )
        # sums for the PREVIOUS kj: its exp() ran while this group's
        # matmuls were executing, so the PE never waits on the scalar engine.
        if kj > 0:
            emit_sums(kj - 1)
        for sc in range(SQC):
            nc.scalar.activation(
                out=att[kj][:, sc * NC:(sc + 1) * NC],
                in_=psums[sc][:], func=Exp, scale=SCALE,
            )
    emit_sums(KT - 1)
    for sc in range(SQC):
        nc.vector.reciprocal(out=recip[:, sc * NC:(sc + 1) * NC],
                             in_=sums_ps[sc][:])

    es_qk.close()  # qt/kt freed after scores

    # ---------------- Phase 2b: aotT[e, qi] = (v.T @ attT) * recip ----------------
    aot = [aot_pool.tile([P, SQ], BF16, tag=f"ao{i}", name=f"ao{i}")
           for i in range(ET)]

    for et in range(ET):
        psums = [ps_mm.tile([P, NC], F32, tag="mm", name="mmp")
                 for _ in range(SQC)]
        for kj in range(KT):
            for sc in range(SQC):
                nc.tensor.matmul(
                    psums[sc][:],
                    vt[kj][:, et * P:(et + 1) * P],
                    att[kj][:, sc * NC:(sc + 1) * NC],
                    start=(kj == 0), stop=(kj == KT - 1),
                )
        for sc in range(SQC):
            nc.vector.tensor_mul(
                aot[et][:, sc * NC:(sc + 1) * NC],
                psums[sc][:],
                recip[:, sc * NC:(sc + 1) * NC],
            )

    es_att.close()  # att freed after attn@v (v stays resident; SBUF fits)

    # ---------------- Phase 2c: outT, gate, result ----------------
    with tc.tile_pool(name="ot", bufs=1) as ot_pool, \
         tc.tile_pool(name="fin", bufs=2) as fin_pool:

        gw = []
        for et in range(ET):
            t = w2_pool.tile([P, E], BF16, tag=f"gw{et}", name=f"gw{et}")
            nc.sync.dma_start(out=t[:], in_=gwT[et * P:(et + 1) * P, :])
            gw.append(t)

        ot = [ot_pool.tile([P, SQ], BF16, tag=f"ot{i}", name=f"ot{i}")
              for i in range(ET)]
        otf = [ot_pool.tile([P, SQ], F32, tag=f"otf{i}", name=f"otf{i}")
               for i in range(ET)]
        for ft in range(ET):
            psums = [ps_mm.tile([P, NC], F32, tag="mm", name="mmp")
                     for _ in range(SQC)]
            for et in range(ET):
                for sc in range(SQC):
                    nc.tensor.matmul(
                        psums[sc][:],
                        wo[et][:, ft * P:(ft + 1) * P],
                        aot[et][:, sc * NC:(sc + 1) * NC],
                        start=(et == 0), stop=(et == ET - 1),
                    )
            for sc in range(SQC):
                nc.vector.tensor_copy(
                    out=ot[ft][:, sc * NC:(sc + 1) * NC], in_=psums[sc][:])
                nc.vector.tensor_copy(
                    out=otf[ft][:, sc * NC:(sc + 1) * NC], in_=psums[sc][:])

        for ft in range(ET):
            psums = [ps_mm.tile([P, NC], F32, tag="mm", name="mmp")
                     for _ in range(SQC)]
            for et in range(ET):
                for sc in range(SQC):
                    nc.tensor.matmul(
                        psums[sc][:],
                        gw[et][:, ft * P:(ft + 1) * P],
                        ot[et][:, sc * NC:(sc + 1) * NC],
                        start=(et == 0), stop=(et == ET - 1),
                    )
            fin = fin_pool.tile([P, SQ], F32, tag="fin")
            for sc in range(SQC):
                gate = fin_pool.tile([P, NC], F32, tag="gate")
                nc.scalar.activation(
                    out=gate[:], in_=psums[sc][:], func=Sigmoid)
                nc.vector.tensor_mul(
                    fin[:, sc * NC:(sc + 1) * NC], gate[:],
                    otf[ft][:, sc * NC:(sc + 1) * NC])
            nc.sync.dma_start(out=outT[ft * P:(ft + 1) * P, :], in_=fin[:])

    w2_es.close()


_NC_CACHE = None


def _get_nc():
    global _NC_CACHE
    if _NC_CACHE is None:
        _NC_CACHE = _build_nc()
    return _NC_CACHE


def _prep_in_maps(rotation_params, entangle_params, inputs, gate_w):
    w_qkv = np.asarray(rotation_params, dtype=np.float32).reshape(3 * E, E)
    wq, wk, wv = w_qkv[:E], w_qkv[E:2 * E], w_qkv[2 * E:]
    w_out = np.asarray(entangle_params, dtype=np.float32).reshape(E, E)
    gw = np.asarray(gate_w, dtype=np.float32)
    x = np.asarray(inputs, dtype=np.float32)

    bf = ml_dtypes.bfloat16
    wqT = np.ascontiguousarray(wq.T).astype(bf)
    wkT = np.ascontiguousarray(wk.T).astype(bf)
    wvT = np.ascontiguousarray(wv.T).astype(bf)
    woT = np.ascontiguousarray(w_out.T).astype(bf)
    gwT = np.ascontiguousarray(gw.T).astype(bf)

    in_maps = []
    for c in range(NCORES):
        b, h = c // 2, c % 2
        xTc = x[b].T  # [E, S]
        if h == 1:   # rotate keys so this core's queries sit at columns 0:SQ
            xTc = np.concatenate([xTc[:, SQ:], xTc[:, :SQ]], axis=1)
        in_maps.append({
            "xT": np.ascontiguousarray(xTc).astype(bf),
            "wqT": wqT, "wkT": wkT, "wvT": wvT, "woT": woT, "gwT": gwT,
        })
    return in_maps


def _assemble(results):
    out = np.empty((B, S, E), dtype=np.float32)
    for c in range(NCORES):
        b, h = c // 2, c % 2
        out[b, h * SQ:(h + 1) * SQ, :] = results[c]["outT"].T
    return out


def _run(in_maps, trace=False):
    nc = _get_nc()
    return run_bass_kernel_spmd(nc, in_maps, core_ids=list(range(NCORES)),
                                trace=trace)


def kernel(rotation_params, entangle_params, inputs, gate_w):
    in_maps = _prep_in_maps(rotation_params, entangle_params, inputs, gate_w)
    res = _run(in_maps, trace=False)
    return _assemble(res.results)


# revision 8
# speedup vs baseline: 1.1751x; 1.0330x over previous
"""Trainium2 Bass kernel for nn_ClassicalSelfAttention (B=4, S=2048, E=1024).

Reference computation (fp32):
    w_qkv = rotation_params.reshape(3E, E); w_out = entangle_params.reshape(E, E)
    qkv = x @ w_qkv.T; q, k, v = split(qkv)
    scores = (q / sqrt(64)) @ k.T          # full-E attention, no heads
    attn = softmax(scores, axis=-1)
    out = (attn @ v) @ w_out.T
    result = sigmoid(out @ gate_w.T) * out

Sharding: 8 cores = 4 batches x 2 query-halves. Each core computes K/V for its
whole batch (duplicated within the pair) and attention + projections for its
1024 queries. Key order is rotated per query-half so each core's queries are
always columns 0:1024 of its (host-pre-transposed) x^T input — softmax and
attn@v are permutation-invariant in key order.

All matmul operands are bf16 (1 col/cycle on the PE, same as f32r, but half
the DMA/SBUF), accumulation in fp32 PSUM. Scores are computed TRANSPOSED
([kj, qi]) so exp() lands directly in the layout attn@v needs — no PE
transposes. Softmax denominators come from an all-ones stationary matmul
(which broadcasts the per-query sums across all 128 partitions for free);
normalization is deferred past attn@v (it is a per-query-column scale, which
commutes with contraction over keys) and applied on the PSUM->SBUF copy.

Layout (feature-major throughout):
    xT [e, s] -> qT [f, s], kT [f, s], v [s, f] (resident in SBUF, no spill)
    scoresT [kj, qi] = kT.T @ qT -> exp -> attT [kj, qi] (bf16)
    sums_bcast [*, qi] = ones.T @ attT (PSUM accum over kj); recip = 1/sums
    aotT [e, qi] = (v.T @ attT) * recip
    outT [f, qi] = woT.T @ aotT;  gateT = gwT.T @ outT
    resultT = sigmoid(gateT) * outT
Host untransposes the per-core [E, 1024] result tiles.
"""

from contextlib import ExitStack

import numpy as np
import ml_dtypes

import concourse.bass as bass
import concourse.tile as tile
from concourse import bacc, mybir
from concourse.bass_utils import run_bass_kernel_spmd

F32 = mybir.dt.float32
BF16 = mybir.dt.bfloat16

P = 128
E = 1024
B = 4
S = 2048
SK = S            # keys per core (full batch sequence)
SQ = S // 2       # queries per core (half)
ET = E // P       # 8 e-tiles
KT = SK // P      # 16 key tiles
NC = 512          # moving-operand chunk (max free dim per PSUM bank)
SKC = SK // NC    # 4
SQC = SQ // NC    # 2
FC = E // NC      # 2
NCORES = 8
SCALE = 1.0 / 8.0  # 1/sqrt(head_dim=64), folded into exp()


def _build_nc():
    nc = bacc.Bacc("TRN2", target_bir_lowering=False, debug=False,
                   num_devices=NCORES)
    xT = nc.dram_tensor("xT", [E, SK], BF16, kind="ExternalInput").ap()
    wqT = nc.dram_tensor("wqT", [E, E], BF16, kind="ExternalInput").ap()
    wkT = nc.dram_tensor("wkT", [E, E], BF16, kind="ExternalInput").ap()
    wvT = nc.dram_tensor("wvT", [E, E], BF16, kind="ExternalInput").ap()
    woT = nc.dram_tensor("woT", [E, E], BF16, kind="ExternalInput").ap()
    gwT = nc.dram_tensor("gwT", [E, E], BF16, kind="ExternalInput").ap()
    outT = nc.dram_tensor("outT", [E, SQ], F32, kind="ExternalOutput").ap()

    with tile.TileContext(nc) as tc, ExitStack() as ctx:
        _emit(tc, ctx, xT, wqT, wkT, wvT, woT, gwT, outT)
    nc.compile()
    return nc


def _emit(tc, ctx, xT, wqT, wkT, wvT, woT, gwT, outT):
    nc = tc.nc
    Exp = mybir.ActivationFunctionType.Exp
    Sigmoid = mybir.ActivationFunctionType.Sigmoid

    singles = ctx.enter_context(tc.tile_pool(name="singles", bufs=1))
    ones = singles.tile([P, P], BF16, tag="ones")
    nc.gpsimd.memset(ones[:], 1.0)

    ps_mm = ctx.enter_context(tc.tile_pool(name="ps_mm", bufs=6, space="PSUM"))

    # Pool stacks are LIFO per side. Left, bottom-up: singles, nrm, v, aot
    # (all ctx-lifetime), then qt/kt (closed after 2a), then phase-1 scratch
    # (closed after phase 1), then 2c scratch. Right: wp2 (ctx-lifetime),
    # then att (closed after 2b).
    nrm_pool = ctx.enter_context(tc.tile_pool(name="nrm", bufs=1))
    recip = nrm_pool.tile([P, SQ], F32, tag="recip")
    v_pool = ctx.enter_context(tc.tile_pool(name="vt", bufs=1))
    vt = [v_pool.tile([P, E], BF16, tag=f"v{i}", name=f"v{i}") for i in range(KT)]
    aot_pool = ctx.enter_context(tc.tile_pool(name="aot", bufs=1))

    es_qk = ExitStack()
    qt_pool = es_qk.enter_context(tc.tile_pool(name="qt", bufs=1))
    kt_pool = es_qk.enter_context(tc.tile_pool(name="kt", bufs=1))
    qt = [qt_pool.tile([P, SQ], BF16, tag=f"qt{i}", name=f"qt{i}") for i in range(ET)]
    kt = [kt_pool.tile([P, SK], BF16, tag=f"kt{i}", name=f"kt{i}") for i in range(ET)]

    # ---------------- Phase 1: qT, v (resident), kT ----------------
    with tc.tile_pool(name="xt", bufs=1) as xt_pool, \
         tc.tile_pool(name="wp", bufs=1) as w_pool:

        # interleaved loads. Full [P, 1024] transfers keep the per-partition
        # line at 2KB (small column chunks crater DMA efficiency). The first
        # qT group needs wq[0..7] + the query half of xt[0..7].
        xt = [xt_pool.tile([P, SK], BF16, tag=f"xt{et}", name=f"xt{et}")
              for et in range(ET)]
        wq = [w_pool.tile([P, E], BF16, tag=f"w{et}", name=f"wq{et}")
              for et in range(ET)]
        for et in range(ET):
            nc.sync.dma_start(out=wq[et][:], in_=wqT[et * P:(et + 1) * P, :])
            nc.sync.dma_start(
                out=xt[et][:, 0:SQ], in_=xT[et * P:(et + 1) * P, 0:SQ])

        # --- qT[f, s] = wqT.T @ xq : groups of 2 ft, accumulate over et
        for sc in range(SQC):
            for fh in range(4):
                psums = [ps_mm.tile([P, NC], F32, tag="mm", name="mmp")
                         for _ in range(2)]
                for et in range(ET):
                    for f2 in range(2):
                        ft = fh * 2 + f2
                        nc.tensor.matmul(
                            psums[f2][:],
                            wq[et][:, ft * P:(ft + 1) * P],
                            xt[et][:, sc * NC:(sc + 1) * NC],
                            start=(et == 0), stop=(et == ET - 1),
                        )
                for f2 in range(2):
                    ft = fh * 2 + f2
                    nc.vector.tensor_copy(
                        out=qt[ft][:, sc * NC:(sc + 1) * NC], in_=psums[f2][:])

        # --- v[s, f] = xT.T @ wvT : stationary xT block, moving wv; resident
        wv = []
        for et in range(ET):
            t = w_pool.tile([P, E], BF16, tag=f"wv{et}", name=f"wv{et}")
            nc.sync.dma_start(out=t[:], in_=wvT[et * P:(et + 1) * P, :])
            wv.append(t)
            nc.sync.dma_start(
                out=xt[et][:, SQ:SK], in_=xT[et * P:(et + 1) * P, SQ:SK])
        wk = []
        for et in range(ET):
            t = w_pool.tile([P, E], BF16, tag=f"wk{et}", name=f"wk{et}")
            nc.sync.dma_start(out=t[:], in_=wkT[et * P:(et + 1) * P, :])
            wk.append(t)
        for st in range(KT):
            psums = [ps_mm.tile([P, NC], F32, tag="mm", name="mmp")
                     for _ in range(FC)]
            for et in range(ET):
                for fc in range(FC):
                    nc.tensor.matmul(
                        psums[fc][:],
                        xt[et][:, st * P:(st + 1) * P],
                        wv[et][:, fc * NC:(fc + 1) * NC],
                        start=(et == 0), stop=(et == ET - 1),
                    )
            for fc in range(FC):
                nc.vector.tensor_copy(
                    out=vt[st][:, fc * NC:(fc + 1) * NC], in_=psums[fc][:])

        # --- kT[f, s] = wkT.T @ xT (full SK columns), groups of 2 kc
        for ft in range(ET):
            for kh in range(2):
                psums = [ps_mm.tile([P, NC], F32, tag="mm", name="mmp")
                         for _ in range(2)]
                for et in range(ET):
                    for k2 in range(2):
                        kc = kh * 2 + k2
                        nc.tensor.matmul(
                            psums[k2][:],
                            wk[et][:, ft * P:(ft + 1) * P],
                            xt[et][:, kc * NC:(kc + 1) * NC],
                            start=(et == 0), stop=(et == ET - 1),
                        )
                for k2 in range(2):
                    kc = kh * 2 + k2
                    nc.vector.tensor_copy(
                        out=kt[ft][:, kc * NC:(kc + 1) * NC], in_=psums[k2][:])

    # ---------------- Phase 2a: scoresT -> exp -> attT; sums via ones ----------------
    w2_es = ExitStack()
    w2_pool = w2_es.enter_context(tc.tile_pool(name="wp2", bufs=1, side="right"))
    wo = []
    for et in range(ET):
        t = w2_pool.tile([P, E], BF16, tag=f"w2{et}", name=f"wo{et}")
        nc.sync.dma_start(out=t[:], in_=woT[et * P:(et + 1) * P, :])
        wo.append(t)

    es_att = ExitStack()
    att_pool = es_att.enter_context(tc.tile_pool(name="att", bufs=1, side="right"))
    att = [att_pool.tile([P, SQ], BF16, tag=f"at{i}", name=f"at{i}")
           for i in range(KT)]

    sums_pool = ctx.enter_context(tc.tile_pool(name="ps_sums", bufs=1, space="PSUM"))
    sums_ps = [sums_pool.tile([P, NC], F32, tag=f"sums{sc}", name=f"sums{sc}")
               for sc in range(SQC)]

    def emit_sums(kj):
        # per-query exp-sums, broadcast to all 128 partitions by all-ones
        # stationary; PSUM-accumulated across all 16 key tiles.
        for sc in range(SQC):
            nc.tensor.matmul(
                sums_ps[sc][:],
                ones[:],
                att[kj][:, sc * NC:(sc + 1) * NC],
                start=(kj == 0), stop=(kj == KT - 1),
            )

    for kj in range(KT):
        psums = [ps_mm.tile([P, NC], F32, tag="mm", name="mmp")
                 for _ in range(SQC)]
        for et in range(ET):
            for sc in range(SQC):
                nc.tensor.matmul(
                    psums[sc][:],
                    kt[et][:, kj * P:(kj + 1) * P],
                    qt[et][:, sc * NC:(sc + 1) * NC],
                    start=(et == 0), stop=(et == ET - 1),
                )
        # sums for the PREVIOUS kj: its exp() ran while this group's
        # matmuls were executing, so the PE never waits on the scalar engine.
        if kj > 0:
            emit_sums(kj - 1)
        for sc in range(SQC):
            nc.scalar.activation(
                out=att[kj][:, sc * NC:(sc + 1) * NC],
                in_=psums[sc][:], func=Exp, scale=SCALE,
            )
    emit_sums(KT - 1)
    for sc in range(SQC):
        nc.vector.reciprocal(out=recip[:, sc * NC:(sc + 1) * NC],
                             in_=sums_ps[sc][:])

    es_qk.close()  # qt/kt freed after scores

    # ---------------- Phase 2b: aotT[e, qi] = (v.T @ attT) * recip ----------------
    aot = [aot_pool.tile([P, SQ], BF16, tag=f"ao{i}", name=f"ao{i}")
           for i in range(ET)]

    for et in range(ET):
        psums = [ps_mm.tile([P, NC], F32, tag="mm", name="mmp")
                 for _ in range(SQC)]
        for kj in range(KT):
            for sc in range(SQC):
                nc.tensor.matmul(
                    psums[sc][:],
                    vt[kj][:, et * P:(et + 1) * P],
                    att[kj][:, sc * NC:(sc + 1) * NC],
                    start=(kj == 0), stop=(kj == KT - 1),
                )
        for sc in range(SQC):
            nc.vector.tensor_mul(
                aot[et][:, sc * NC:(sc + 1) * NC],
                psums[sc][:],
                recip[:, sc * NC:(sc + 1) * NC],
            )

    es_att.close()  # att freed after attn@v (v stays resident; SBUF fits)

    # ---------------- Phase 2c: outT, gate, result ----------------
    with tc.tile_pool(name="ot", bufs=1) as ot_pool, \
         tc.tile_pool(name="fin", bufs=2) as fin_pool:

        gw = []
        for et in range(ET):
            t = w2_pool.tile([P, E], BF16, tag=f"gw{et}", name=f"gw{et}")
            nc.sync.dma_start(out=t[:], in_=gwT[et * P:(et + 1) * P, :])
            gw.append(t)

        ot = [ot_pool.tile([P, SQ], BF16, tag=f"ot{i}", name=f"ot{i}")
              for i in range(ET)]
        otf = [ot_pool.tile([P, SQ], F32, tag=f"otf{i}", name=f"otf{i}")
               for i in range(ET)]
        for ft in range(ET):
            psums = [ps_mm.tile([P, NC], F32, tag="mm", name="mmp")
                     for _ in range(SQC)]
            for et in range(ET):
                for sc in range(SQC):
                    nc.tensor.matmul(
                        psums[sc][:],
                        wo[et][:, ft * P:(ft + 1) * P],
                        aot[et][:, sc * NC:(sc + 1) * NC],
                        start=(et == 0), stop=(et == ET - 1),
                    )
            for sc in range(SQC):
                nc.vector.tensor_copy(
                    out=ot[ft][:, sc * NC:(sc + 1) * NC], in_=psums[sc][:])
                nc.vector.tensor_copy(
                    out=otf[ft][:, sc * NC:(sc + 1) * NC], in_=psums[sc][:])

        for ft in range(ET):
            psums = [ps_mm.tile([P, NC], F32, tag="mm", name="mmp")
                     for _ in range(SQC)]
            for et in range(ET):
                for sc in range(SQC):
                    nc.tensor.matmul(
                        psums[sc][:],
                        gw[et][:, ft * P:(ft + 1) * P],
                        ot[et][:, sc * NC:(sc + 1) * NC],
                        start=(et == 0), stop=(et == ET - 1),
                    )
            fin = fin_pool.tile([P, SQ], F32, tag="fin")
            for sc in range(SQC):
                gate = fin_pool.tile([P, NC], F32, tag="gate")
                nc.scalar.activation(
                    out=gate[:], in_=psums[sc][:], func=Sigmoid)
                nc.vector.tensor_mul(
                    fin[:, sc * NC:(sc + 1) * NC], gate[:],
                    otf[ft][:, sc * NC:(sc + 1) * NC])
            nc.sync.dma_start(out=outT[ft * P:(ft + 1) * P, :], in_=fin[:])

    w2_es.close()


_NC_CACHE = None


def _get_nc():
    global _NC_CACHE
    if _NC_CACHE is None:
        _NC_CACHE = _build_nc()
    return _NC_CACHE


def _prep_in_maps(rotation_params, entangle_params, inputs, gate_w):
    w_qkv = np.asarray(rotation_params, dtype=np.float32).reshape(3 * E, E)
    wq, wk, wv = w_qkv[:E], w_qkv[E:2 * E], w_qkv[2 * E:]
    w_out = np.asarray(entangle_params, dtype=np.float32).reshape(E, E)
    gw = np.asarray(gate_w, dtype=np.float32)
    x = np.asarray(inputs, dtype=np.float32)

    bf = ml_dtypes.bfloat16
    wqT = np.ascontiguousarray(wq.T).astype(bf)
    wkT = np.ascontiguousarray(wk.T).astype(bf)
    wvT = np.ascontiguousarray(wv.T).astype(bf)
    woT = np.ascontiguousarray(w_out.T).astype(bf)
    gwT = np.ascontiguousarray(gw.T).astype(bf)

    in_maps = []
    for c in range(NCORES):
        b, h = c // 2, c % 2
        xTc = x[b].T  # [E, S]
        if h == 1:   # rotate keys so this core's queries sit at columns 0:SQ
            xTc = np.concatenate([xTc[:, SQ:], xTc[:, :SQ]], axis=1)
        in_maps.append({
            "xT": np.ascontiguousarray(xTc).astype(bf),
            "wqT": wqT, "wkT": wkT, "wvT": wvT, "woT": woT, "gwT": gwT,
        })
    return in_maps


def _assemble(results):
    out = np.empty((B, S, E), dtype=np.float32)
    for c in range(NCORES):
        b, h = c // 2, c % 2
        out[b, h * SQ:(h + 1) * SQ, :] = results[c]["outT"].T
    return out


def _run(in_maps, trace=False):
    nc = _get_nc()
    return run_bass_kernel_spmd(nc, in_maps, core_ids=list(range(NCORES)),
                                trace=trace)


def kernel(rotation_params, entangle_params, inputs, gate_w):
    in_maps = _prep_in_maps(rotation_params, entangle_params, inputs, gate_w)
    res = _run(in_maps, trace=False)
    return _assemble(res.results)
